# revision 47
# baseline (speedup 1.0000x reference)
"""Trainium2 Bass kernel for the MemoryReader (retrieval-knn) module.

Math (per batch b):
    a[m]     = sum_ck mk[ck, m]^2
    logits   = (2 * mk^T qk - a) / sqrt(CK)        # [THW, NQ]
    aff      = softmax(logits, axis=THW)
    out      = mv @ aff                            # [CV, NQ]

Shapes: B=4, CK=64, T=8, H=30, W=54 (THW=12960, NQ=1620), CV=512.

Sharding: 8 cores = (B=4) x (NQ halves of 810).  Softmax is over THW,
which every core owns fully, so no cross-core reduction is needed.

The squared-norm term is folded into the score matmul by augmenting the
contraction dim to K=128:
    lhsT' = [mk ; mk^2]  (host-prepared, [128, THW])
    rhs'  = [qk ; -0.5 ]  (host-prepared, [128, 810])
    psum  = mk.qk - a/2  ->  logits = 0.25 * psum  (ACT scale)
All exps carry a global bias EXP_BIAS (exp(0.25 x + 1.5)); the factor
e^1.5 cancels in the final normalization and pushes the fp8 exp tiles
(below) up into e4m3's well-covered range.

Performance structure (per core, cost-model-driven; ~155.5us vs the
183.2us all-bf16 predecessor):
  * Matmul operands bf16 except: 41 of the 102 m-tiles (evenly spread
    over [20, 98) so ACT's exp keeps pace and block edges stay bf16) run
    the readout in fp8e4 DoubleRow mode (0.5 cycles/col): lhsT holds
    (mv_hi, mv_lo) e4m3 splits of mv in the two DR planes (host-
    prepared, mv numerically exact to ~0.2%), rhs is the fp8 exp tile
    broadcast (stride-0) into both planes.  Only the exp-side e4m3
    rounding of those tiles adds error; measured rel err 1.77e-2 vs the
    2e-2 gate (42 tiles measured 1.82e-2 -- the slope is super-linear;
    41 keeps ~11% gate headroom).  (An error-free 3-product residual variant exists behind
    the pattern table -- 'R' tiles -- but its DVE hi/lo-split latency
    stalls the PE more than the saved cycles; left dormant.)
  * All readouts are emitted with a uniform two-iteration lag behind
    their tile's score/exp so cross-engine chain latency never stalls
    the PE; scores run two tiles ahead (three psum score banks).
  * THW zero-padded to 13056 = 102*128; pad tokens carry a poison value
    in the norm channel driving logits to -80 (exp -> 0).
  * mv is DMA'd ONCE into resident SBUF (bf16 tiles + fp8 hi/lo pairs,
    1KiB/partition/tile either way) and reused by both query blocks.
  * Scores are software-pipelined two m-tiles ahead of the readout.
  * Startup: the first mkq chunks ride the ACT and DVE DGE queues so
    they don't serialize behind the SP queue (650ns per issue); mv8
    rides the Pool SWDGE queue.  Junk matmuls on DVE-memset SBUF keep
    the PE busy (and burn the p-state ramp) exactly until the head DMA
    lands.
  * Query blocks (512, 298): psum-bank-sized; the small block last
    shortens the endgame.  Outputs are processed as bank-PAIRS: the acc
    psum is allocated as two [P, 2, 512] tiles, normalized by two
    grouped DVE muls against the PE-broadcast recip row, and leave as
    two 2-row-per-partition DMAs on different DGE queues.
  * Denominator in fp16 (2x DVE mode, ~8x less rounding than bf16);
    fp8 tiles accumulate via mixed fp8+fp16 DVE adds; the last tile's
    exp + den-tail + recip + bcast are hoisted a tile early so the
    normalization chain overlaps the final readouts; block 0's output
    is evacuated to fp16 at the boundary and normalized later in block
    1's DVE slack.
"""

import os
import sys

import numpy as np
import ml_dtypes

for _p in ("/opt/trn_rl_repo",):
    if _p not in sys.path and os.path.isdir(_p):
        sys.path.insert(0, _p)

B, CK, T, H, W = 4, 64, 8, 30, 54
CV = 512
THW = T * H * W          # 12960
NQ = H * W               # 1620
QH = NQ // 2             # 810   per-core query half
QBLKS = [(0, 512), (512, 298)]  # two query passes (psum slice padded to 512)
P = 128
NT = 102                 # padded m-tiles
THWP = NT * P            # 13056
# Tile kinds over the 78 mid tiles [20, 98): N = fp8 DoubleRow with e4m3
# exp (cheap, noisy); R = fp8 DoubleRow 3-product residual pairs (cheap,
# accurate, needs DVE prep); B = bf16.  Edge tiles stay bf16.
_P13 = "RRNNBNRRNNBNN"   # R4 N7 B2
_P13c = "RRNBNBRRNBNNB"  # R4 N5 B4
_PATTERN = _P13 + _P13 + _P13c + _P13 + _P13 + _P13c
if 1:  # KR0 experiment: 38 N spread evenly, no residual tiles
    _pos = sorted(int((i + 0.5) * 78 / 41) for i in range(41))
    _PATTERN = "".join("N" if p in _pos else "B" for p in range(78))
assert len(_PATTERN) == 78
KIND_BY_TILE = ["B"] * 20 + list(_PATTERN) + ["B"] * 4
K8 = KIND_BY_TILE.count("N")   # 38 noisy fp8 tiles
KR = KIND_BY_TILE.count("R")   # 24 residual fp8 tiles (12 pairs)
NB = KIND_BY_TILE.count("B")   # 40 bf16 m-tiles
N8SLOT = 2 * K8 + 5 * (KR // 2)  # mv8 slots: (hi,lo) / (hiA,loA,hiA,hiB,loB)
PAD_POISON = 640.0       # pad-token norm channel: psum=-320 -> logit=-80
EXP_BIAS = 1.5           # exp(0.25x + 1.5); e^1.5 cancels in normalization
CH = 3                   # mkq m-tiles per DMA chunk (102 = 34*3)
NCHUNK = NT // CH        # 34
JUNK_128 = 5             # PE-warmup junk matmuls of 128 cols (early)
JUNK_512 = 5             # PE-warmup junk matmuls of NQ0 cols

# tile index -> storage slot.  B: index into mvp; N: base of its (hi,lo)
# pair in mv8p; R (first of pair): base of the 5-slot group; R (second):
# base + 3 (its (hi,lo) sit at +3,+4; the cross instr reads +2,+3).
_SLOT = [0] * NT
_RFIRST = [False] * NT
_nb = _n8 = 0
_pend = None
for _t in range(NT):
    k = KIND_BY_TILE[_t]
    if k == "B":
        _SLOT[_t] = _nb
        _nb += 1
    elif k == "N":
        _SLOT[_t] = _n8
        _n8 += 2
    else:
        if _pend is None:
            _pend = _t
            _RFIRST[_t] = True
            _SLOT[_t] = _n8
        else:
            _SLOT[_t] = _n8 + 3
            _n8 += 5
            _pend = None
assert _pend is None and _nb == NB and _n8 == N8SLOT

_PROGRAM = None
_BF16 = ml_dtypes.bfloat16
_FP8 = ml_dtypes.float8_e4m3fn


def _build_program():
    import concourse.mybir as mybir
    import concourse.tile as tile
    from concourse import bacc

    f32 = mybir.dt.float32
    f32r = mybir.dt.float32r
    bf16 = mybir.dt.bfloat16
    fp16 = mybir.dt.float16
    fp8 = mybir.dt.float8e4
    Exp = mybir.ActivationFunctionType.Exp
    DR = mybir.MatmulPerfMode.DoubleRow

    nc = bacc.Bacc(
        "TRN2",
        target_bir_lowering=False,
        debug=False,
        enable_asserts=False,
        num_devices=8,
    )

    mkq = nc.dram_tensor("mkq", [P, THWP], bf16, kind="ExternalInput").ap()
    qkc = nc.dram_tensor("qkc", [P, QH], bf16, kind="ExternalInput").ap()
    NQ0 = QBLKS[0][1]
    PSW = 512             # bank-aligned pair-slice width
    HEADC = NQ0 + CH * P  # qkc block-0 + mkq chunk 0
    head = nc.dram_tensor("head", [P, HEADC], bf16, kind="ExternalInput").ap()
    mvp = nc.dram_tensor("mvp", [P, NB, CV], bf16, kind="ExternalInput").ap()
    mv8p = nc.dram_tensor("mv8p", [P, N8SLOT, CV], fp8, kind="ExternalInput").ap()
    out = nc.dram_tensor("out", [CV, QH], bf16, kind="ExternalOutput").ap()

    NVT = 2 * NT  # virtual tiles: (block, m-tile) flattened

    with tile.TileContext(nc) as tc:
        with (
            tc.tile_pool(name="const", bufs=1) as cpool,
            tc.tile_pool(name="exp", bufs=10) as expool,
            tc.tile_pool(name="exp8", bufs=8) as ex8pool,
            tc.tile_pool(name="den", bufs=2) as dpool,
            tc.tile_pool(name="vec", bufs=2) as vpool,
            tc.tile_pool(name="outp", bufs=4) as opool,
            tc.tile_pool(name="wlo", bufs=3) as wlopool,
            tc.tile_pool(name="score_ps", bufs=3, space="PSUM") as spspool,
            tc.tile_pool(name="acc_ps", bufs=1, space="PSUM") as apspool,
            tc.tile_pool(name="den_ps", bufs=1, space="PSUM") as dpspool,
        ):
            # PE warm-up: junk matmuls on DVE-memset SBUF burn the p-state
            # ramp until the head DMA lands.  The first few use the small
            # junk_w tile as both operands so they can start the moment its
            # memset finishes.
            junk_w = cpool.tile([P, P], bf16, tag="junk_w", name="junk_w")
            nc.gpsimd.memset(junk_w[:], 0.0)
            junk_r = cpool.tile([P, NQ0], bf16, tag="junk_r", name="junk_r")
            # on the Pool queue: also delays mv01's SWDGE gen just enough
            # that mkq1 wins the serial-DMA slot after the head transfer.
            nc.gpsimd.memset(junk_r[:], 0.0)
            for _ in range(JUNK_128):
                jp = spspool.tile([P, NQ0], f32, tag="score", name="warm")
                nc.tensor.matmul(
                    jp[:, :P], lhsT=junk_w[:], rhs=junk_w[:], start=True, stop=True
                )
            for _ in range(JUNK_512):
                jp = spspool.tile([P, NQ0], f32, tag="score", name="warm")
                nc.tensor.matmul(
                    jp[:], lhsT=junk_w[:], rhs=junk_r[:], start=True, stop=True
                )

            head_sb = cpool.tile([P, HEADC], bf16, tag="head", name="head")
            mkq_sb = cpool.tile([P, THWP - CH * P], bf16, tag="mkq", name="mkq")
            qkc_sb = cpool.tile([P, QH - NQ0], bf16, tag="qkc", name="qkc")
            mv_sb = cpool.tile([P, NB * CV], bf16, tag="mv", name="mv")
            mv8_sb = cpool.tile([P, N8SLOT, CV], fp8, tag="mv8", name="mv8")
            # Whole head (block-0 queries + mkq tiles 0-2) in one SP DMA;
            # mkq chunks 1-2 ride the ACT/DVE queues so they don't serialize
            # behind SP's 650ns-per-issue; mv tiles 0-1 + mv8 ride Pool SWDGE.
            # Need-ordered DMA stream.  The cost model serializes transfers on
            # one DMA resource, so order = priority: head (tiles 0-2 +
            # queries), then mkq chunks strictly ahead of the mv stream;
            # mv tiles 0-1 ride Pool SWDGE (its descriptor gen overlaps the
            # head transfer); mv8 goes in quarters so it never blocks mkq.
            nc.sync.dma_start(out=head_sb[:], in_=head[:])
            nc.sync.dma_start(
                out=mkq_sb[:, : CH * P], in_=mkq[:, CH * P : 2 * CH * P]
            )
            nc.sync.dma_start(out=mv_sb[:, : 2 * CV], in_=mvp[:, :2, :])
            nc.sync.dma_start(
                out=mkq_sb[:, CH * P : 2 * CH * P], in_=mkq[:, 2 * CH * P : 3 * CH * P]
            )
            Q8 = (N8SLOT + 7) // 8  # mv8 eighth (in slots)
            vnext = 2
            for i in range(3, NCHUNK):
                k0, k1 = i * CH * P, (i + 1) * CH * P
                nc.sync.dma_start(
                    out=mkq_sb[:, k0 - CH * P : k1 - CH * P], in_=mkq[:, k0:k1]
                )
                if i == 4:
                    q1, nq1 = QBLKS[1]
                    nc.sync.dma_start(out=qkc_sb[:], in_=qkc[:, q1 : q1 + nq1])
                if i in (6, 9, 12, 15, 18, 21, 24, 27):
                    q = (6, 9, 12, 15, 18, 21, 24, 27).index(i)
                    s0 = Q8 * q
                    s1 = Q8 * (q + 1) if q < 7 else N8SLOT
                    nc.sync.dma_start(
                        out=mv8_sb[:, s0:s1, :], in_=mv8p[:, s0:s1, :]
                    )
                v1 = min(vnext + 2, NB)
                if vnext < v1:
                    nc.sync.dma_start(
                        out=mv_sb[:, vnext * CV : v1 * CV], in_=mvp[:, vnext:v1, :]
                    )
                    vnext = v1
            if vnext < NB:
                nc.sync.dma_start(
                    out=mv_sb[:, vnext * CV :], in_=mvp[:, vnext:, :]
                )

            bias_sb = cpool.tile([P, 1], f32, tag="bias", name="bias")
            nc.vector.memset(bias_sb[:], EXP_BIAS)
            ones_col_f = cpool.tile([P, 1], f32, tag="ones_col_f", name="ones_col_f")
            nc.vector.memset(ones_col_f[:], 1.0)
            ones_col = cpool.tile([P, 1], f32r, tag="ones_col", name="ones_col")
            with nc.allow_low_precision("exact 1.0 cast to f32r"):
                nc.vector.tensor_copy(ones_col[:], ones_col_f[:])
            ones_col_b = cpool.tile([P, 1], bf16, tag="ones_col_b", name="ones_col_b")
            nc.vector.memset(ones_col_b[:], 1.0)
            ones_col_h = cpool.tile([P, 1], fp16, tag="ones_col_h", name="ones_col_h")
            nc.vector.memset(ones_col_h[:], 1.0)
            ones_row_f = cpool.tile([1, P], f32, tag="ones_row_f", name="ones_row_f")
            nc.vector.memset(ones_row_f[:], 1.0)
            ones_row = cpool.tile([1, P], f32r, tag="ones_row", name="ones_row")
            with nc.allow_low_precision("exact 1.0 cast to f32r"):
                nc.vector.tensor_copy(ones_row[:], ones_row_f[:])

            # Per-block state created lazily at block entry.
            rstate = [None]  # current residual pair's w_lo buffer
            accs = [None, None]   # each: two [P, 2, 512] psum bank-pairs
            dens = [None, None]
            scores = [None] * NVT

            def vt_block(j):
                return 0 if j < NT else 1

            def emit_score(j):
                blk = vt_block(j)
                q0, nq = QBLKS[blk]
                mi = j - blk * NT
                if mi < CH:
                    lhsT = head_sb[:, NQ0 + mi * P : NQ0 + (mi + 1) * P]
                else:
                    lhsT = mkq_sb[:, (mi - CH) * P : (mi - CH + 1) * P]
                rhs = head_sb[:, :NQ0] if blk == 0 else qkc_sb[:]
                s = spspool.tile([P, nq], f32, tag="score", name="score")
                nc.tensor.matmul(s[:], lhsT=lhsT, rhs=rhs, start=True, stop=True)
                scores[j] = s

            def emit_block_entry_dens(blk):
                d = dpool.tile([P, QBLKS[blk][1]], fp16, tag="den", name="den")
                nc.vector.memset(d[:], 0.0)
                dens[blk] = d

            def emit_block_entry_accs(blk):
                accs[blk] = [
                    apspool.tile([P, 2, PSW], f32, tag=f"accp{p}", name=f"accp{p}")
                    for p in range(2)
                ]

            recips = [None, None]
            den_sums = [None, None]

            def emit_den_main(blk):
                # Partial denominator (tiles 0..NT-2) reduced on PE right
                # after the second-to-last DVE add -- off the endgame chain.
                nq = QBLKS[blk][1]
                den_sum = dpspool.tile([1, nq], f32, tag="den_sum", name="den_sum")
                nc.tensor.matmul(
                    den_sum[:],
                    lhsT=ones_col_h[:],
                    rhs=dens[blk][:],
                    start=True,
                    stop=False,
                )
                den_sums[blk] = den_sum

            def emit_den_tail_recip(blk, ex_last):
                # Last tile's exp summed straight off ACT's output on the PE,
                # skipping the last DVE accumulator add on the endgame chain.
                nq = QBLKS[blk][1]
                nc.tensor.matmul(
                    den_sums[blk][:],
                    lhsT=ones_col_b[:],
                    rhs=ex_last[:],
                    start=False,
                    stop=True,
                )
                r = vpool.tile([1, nq], f32r, tag="recip", name="recip")
                with nc.allow_low_precision("feeds f32r broadcast matmul"):
                    nc.vector.reciprocal(r[:], den_sums[blk][:])
                recips[blk] = r

            evacs = [None, None]

            def emit_evac(blk):
                # ACT evacuates the two acc bank-pairs to fp16 SBUF, freeing
                # the psum banks for the next block ~2.5us sooner than
                # normalize-in-place; normalization happens later during the
                # next block's DVE slack.
                q0, nq = QBLKS[blk]
                evacs[blk] = []
                for pair in range(2):
                    e = cpool.tile([P, 2, PSW], fp16, tag=f"evac{pair}", name=f"evac{pair}")
                    # per-slice copies: subtile deps let the next block's
                    # readout of chunk c start as soon as ITS slice is free.
                    for k in range(2):
                        nc.scalar.copy(e[:, k, :nq], accs[blk][pair][:, k, :nq])
                    evacs[blk].append(e)

            def emit_norm_deferred(blk):
                q0, nq = QBLKS[blk]
                bcast_ps = spspool.tile([P, nq], f32, tag="score", name="bcast")
                nc.tensor.matmul(
                    bcast_ps[:],
                    lhsT=ones_row[:],
                    rhs=recips[blk][:],
                    start=True,
                    stop=True,
                )
                bcast_sb = vpool.tile([P, nq], bf16, tag="bcast_sb", name="bcast_sb")
                nc.vector.tensor_copy(bcast_sb[:], bcast_ps[:])
                rep = bcast_sb[:].unsqueeze(1).broadcast_to([P, 2, nq])
                engs = [nc.sync, nc.gpsimd]
                for pair in range(2):
                    o = opool.tile([P, 2, PSW], bf16, tag="out", name="out")
                    nc.vector.tensor_mul(
                        o[:, :, :nq], evacs[blk][pair][:, :, :nq], rep
                    )
                    dst = out[
                        pair * 2 * P : (pair + 1) * 2 * P, q0 : q0 + nq
                    ].rearrange("(two p) q -> p two q", two=2)
                    engs[pair].dma_start(out=dst, in_=o[:, :, :nq])

            def emit_bcast(blk):
                # bcast the per-query recip onto 128 partitions (PE), copy it
                # to SBUF (ACT).  Emitted BEFORE the last tile's readouts so
                # this chain overlaps them.
                q0, nq = QBLKS[blk]
                bcast_ps = spspool.tile([P, nq], f32, tag="score", name="bcast")
                nc.tensor.matmul(
                    bcast_ps[:],
                    lhsT=ones_row[:],
                    rhs=recips[blk][:],
                    start=True,
                    stop=True,
                )
                bcast_sb = vpool.tile([P, nq], bf16, tag="bcast_sb", name="bcast_sb")
                nc.scalar.copy(bcast_sb[:], bcast_ps[:])
                return bcast_sb

            def emit_muls(blk, bcast_sb):
                # Normalize each acc bank-PAIR in one grouped DVE mul and
                # ship it as one 2-row-per-partition DMA.
                q0, nq = QBLKS[blk]
                rep = bcast_sb[:].unsqueeze(1).broadcast_to([P, 2, nq])
                engs = [nc.sync, nc.sync]
                for pair in range(2):
                    o = opool.tile([P, 2, PSW], bf16, tag="out", name="out")
                    nc.vector.tensor_mul(
                        o[:, :, :nq], accs[blk][pair][:, :, :nq], rep
                    )
                    dst = out[
                        pair * 2 * P : (pair + 1) * 2 * P, q0 : q0 + nq
                    ].rearrange("(two p) q -> p two q", two=2)
                    engs[pair].dma_start(out=dst, in_=o[:, :, :nq])

            def _dr4(blk, nq, base, rhs, start, stop):
                for c in range(4):
                    nc.tensor.matmul(
                        accs[blk][c // 2][:, c % 2, :nq],
                        lhsT=mv8_sb[:, base : base + 2, c * P : (c + 1) * P],
                        rhs=rhs,
                        start=start,
                        stop=stop,
                        perf_mode=DR,
                    )

            def emit_readout(j, ex, whi=None, wlo=None):
                blk = vt_block(j)
                mi = j - blk * NT
                nq = QBLKS[blk][1]
                kind = KIND_BY_TILE[mi]
                sl = _SLOT[mi]
                if kind == "B":
                    def _ro(blk=blk, nq=nq, sl=sl, mi=mi, ex=ex):
                        for c in range(4):
                            nc.tensor.matmul(
                                accs[blk][c // 2][:, c % 2, :nq],
                                lhsT=mv_sb[
                                    :, sl * CV + c * P : sl * CV + (c + 1) * P
                                ],
                                rhs=ex[:],
                                start=(mi == 0),
                                stop=(mi == NT - 1),
                            )
                elif kind == "N":
                    def _ro(blk=blk, nq=nq, sl=sl, mi=mi, ex=ex):
                        rhs = ex[:].unsqueeze(1).broadcast_to([P, 2, nq])
                        _dr4(blk, nq, sl, rhs, mi == 0, mi == NT - 1)
                else:
                    def _ro(blk=blk, nq=nq, sl=sl, whi=whi):
                        rhs = whi[:].unsqueeze(1).broadcast_to([P, 2, nq])
                        _dr4(blk, nq, sl, rhs, False, False)
                    if not _RFIRST[mi]:
                        # second of pair: the cross instr (hiA, hiB) x
                        # (w_lo_a, w_lo_b) waits on the DVE lo-split; give it
                        # one more iteration of slack.
                        def _rx(blk=blk, nq=nq, sl=sl, wlo=wlo):
                            _dr4(blk, nq, sl - 1, wlo[:, :, :nq], False, False)
                        pending.append((j + 3, _rx))
                pending.append((j + 2, _ro))

            emit_block_entry_dens(0)
            emit_block_entry_accs(0)
            emit_score(0)
            emit_score(1)
            pending = []
            for j in range(NVT):
                if j == NT:
                    for jj in range(j + 2, j + 6):
                        emit_score(jj)
                elif j + 2 < NVT and not (NT < j < NT + 4):
                    emit_score(j + 2)
                if j == NT + 1:
                    # block 1's acc psum rotates in only now: every access to
                    # block 0's acc tiles has been emitted.
                    emit_block_entry_accs(1)
                for item in pending[:]:
                    if item[0] <= j:
                        item[1]()
                        pending.remove(item)
                blk = vt_block(j)
                if j == NT:
                    emit_block_entry_dens(1)
                q0, nq = QBLKS[blk]
                mi = j - blk * NT
                if mi == NT - 1 and j == NVT - 1:
                    # exp/den-tail/recip were hoisted into the previous
                    # iteration: flush r/o[NVT-2], then bcast, then the last
                    # readouts inline so the bcast chain overlaps them.
                    for item in pending:
                        item[1]()
                    pending.clear()
                    final_bcast = emit_bcast(1)
                    for c in range(4):
                        nc.tensor.matmul(
                            accs[blk][c // 2][:, c % 2, :nq],
                            lhsT=mv_sb[
                                :,
                                _SLOT[mi] * CV + c * P : _SLOT[mi] * CV
                                + (c + 1) * P,
                            ],
                            rhs=hoisted_ex[:],
                            start=False,
                            stop=True,
                        )
                    continue
                kind = KIND_BY_TILE[mi]
                ex = (
                    ex8pool.tile([P, nq], fp8, tag="exp8", name="exp8")
                    if kind == "N"
                    else expool.tile([P, nq], bf16, tag="exp", name="exp")
                )
                nc.scalar.activation(
                    ex[:], scores[j][:], Exp, bias=bias_sb[:], scale=0.25
                )
                scores[j] = None
                whi = wlo = None
                if kind == "R":
                    whi = ex8pool.tile([P, nq], fp8, tag="exp8", name="whi")
                    if _RFIRST[mi]:
                        rstate[0] = wlopool.tile(
                            [P, 2, PSW], fp8, tag="wlo", name="wlo"
                        )
                    wlo = rstate[0]
                    plane = 0 if _RFIRST[mi] else 1
                    with nc.allow_low_precision("fp8 exp hi/lo split"):
                        nc.vector.tensor_copy(whi[:], ex[:])
                        nc.vector.scalar_tensor_tensor(
                            wlo[:, plane, :nq],
                            ex[:],
                            1.0,
                            whi[:],
                            mybir.AluOpType.mult,
                            mybir.AluOpType.subtract,
                        )
                if mi < NT - 1:
                    with nc.allow_low_precision("den feeds f32r den_sum matmul"):
                        nc.vector.tensor_add(dens[blk][:], dens[blk][:], ex[:])
                    if mi == NT - 2:
                        emit_den_main(blk)
                        if j == NVT - 2:
                            # Hoist the final tile's exp + den tail + recip so
                            # the bcast chain overlaps the last 8 readouts.
                            hoisted_ex = expool.tile(
                                [P, nq], bf16, tag="exp", name="exp"
                            )
                            nc.scalar.activation(
                                hoisted_ex[:],
                                scores[NVT - 1][:],
                                Exp,
                                bias=bias_sb[:],
                                scale=0.25,
                            )
                            scores[NVT - 1] = None
                            emit_den_tail_recip(blk, hoisted_ex)
                else:
                    emit_den_tail_recip(blk, ex)
                if j == NT:
                    # Block 1's outputs: flush block 0's remaining readouts,
                    # evacuate (fast bank free), normalize later during block
                    # 1's slack.
                    for item in pending:
                        item[1]()
                    pending.clear()
                    emit_evac(0)
                if j == NT + 8:
                    emit_norm_deferred(0)
                emit_readout(j, ex, whi, wlo)
            emit_muls(1, final_bcast)

    nc.compile()
    return nc


def _get_program():
    global _PROGRAM
    if _PROGRAM is None:
        _PROGRAM = _build_program()
    return _PROGRAM


def _make_in_maps(mk, qk, mv):
    mkf = np.asarray(mk, dtype=np.float32).reshape(B, CK, THW)
    qkf = np.asarray(qk, dtype=np.float32).reshape(B, CK, NQ)
    mvf = np.asarray(mv, dtype=np.float32).reshape(B, CV, THW)

    bf_idx = [t for t in range(NT) if KIND_BY_TILE[t] == "B"]

    in_maps = []
    for b in range(B):
        mkq_b = np.zeros((P, THWP), dtype=_BF16)
        mkq_b[:CK, :THW] = mkf[b]
        mkq_b[CK:, :THW] = mkf[b] * mkf[b]
        mkq_b[CK, THW:] = PAD_POISON  # pad tokens -> logit -80 -> exp ~ 0

        mvt = np.zeros((THWP, CV), dtype=np.float32)
        mvt[:THW] = mvf[b].T
        mvp_full = np.ascontiguousarray(mvt.reshape(NT, P, CV).transpose(1, 0, 2))
        mvp_b = mvp_full[:, bf_idx, :].astype(_BF16)
        hi_all = mvp_full.astype(_FP8)
        lo_all = (mvp_full - hi_all.astype(np.float32)).astype(_FP8)
        mv8_b = np.zeros((P, N8SLOT, CV), dtype=_FP8)
        for t in range(NT):
            k = KIND_BY_TILE[t]
            s = _SLOT[t]
            if k == "N":
                mv8_b[:, s] = hi_all[:, t]
                mv8_b[:, s + 1] = lo_all[:, t]
            elif k == "R" and _RFIRST[t]:
                mv8_b[:, s] = hi_all[:, t]      # hiA
                mv8_b[:, s + 1] = lo_all[:, t]  # loA
                mv8_b[:, s + 2] = hi_all[:, t]  # hiA dup (cross instr)
            elif k == "R":
                mv8_b[:, s] = hi_all[:, t]      # hiB (slot base+3)
                mv8_b[:, s + 1] = lo_all[:, t]  # loB

        for h in range(2):
            qkc_b = np.empty((P, QH), dtype=_BF16)
            qkc_b[:CK] = qkf[b][:, h * QH : (h + 1) * QH]
            qkc_b[CK:] = -0.5
            nq0 = QBLKS[0][1]
            head_b = np.concatenate([qkc_b[:, :nq0], mkq_b[:, : CH * P]], axis=1)
            in_maps.append(
                {
                    "mkq": mkq_b,
                    "qkc": qkc_b,
                    "mvp": mvp_b,
                    "mv8p": mv8_b,
                    "head": head_b,
                }
            )
    return in_maps


def kernel(mk, qk, mv, _trace=False, _results_out=None):
    from concourse import bass_utils

    nc = _get_program()
    in_maps = _make_in_maps(mk, qk, mv)
    res = bass_utils.run_bass_kernel_spmd(
        nc, in_maps, core_ids=list(range(8)), trace=_trace
    )
    if _results_out is not None:
        _results_out.append(res)

    full = np.empty((B, CV, NQ), dtype=np.float32)
    for b in range(B):
        for h in range(2):
            full[b][:, h * QH : (h + 1) * QH] = np.asarray(
                res.results[2 * b + h]["out"], dtype=np.float32
            )
    return full.reshape(B, CV, H, W)


# revision 48
# speedup vs baseline: 1.0051x; 1.0051x over previous
"""Trainium2 Bass kernel for the MemoryReader (retrieval-knn) module.

Math (per batch b):
    a[m]     = sum_ck mk[ck, m]^2
    logits   = (2 * mk^T qk - a) / sqrt(CK)        # [THW, NQ]
    aff      = softmax(logits, axis=THW)
    out      = mv @ aff                            # [CV, NQ]

Shapes: B=4, CK=64, T=8, H=30, W=54 (THW=12960, NQ=1620), CV=512.

Sharding: 8 cores = (B=4) x (NQ halves of 810).  Softmax is over THW,
which every core owns fully, so no cross-core reduction is needed.

The squared-norm term is folded into the score matmul by augmenting the
contraction dim to K=128:
    lhsT' = [mk ; mk^2]  (host-prepared, [128, THW])
    rhs'  = [qk ; -0.5 ]  (host-prepared, [128, 810])
    psum  = mk.qk - a/2  ->  logits = 0.25 * psum  (ACT scale)
All exps carry a global bias EXP_BIAS (exp(0.25 x + 1.5)); the factor
e^1.5 cancels in the final normalization and pushes the fp8 exp tiles
(below) up into e4m3's well-covered range.

Performance structure (per core, cost-model-driven; ~155.5us vs the
183.2us all-bf16 predecessor):
  * Matmul operands bf16 except: 41 of the 102 m-tiles (evenly spread
    over [20, 98) so ACT's exp keeps pace and block edges stay bf16) run
    the readout in fp8e4 DoubleRow mode (0.5 cycles/col): lhsT holds
    (mv_hi, mv_lo) e4m3 splits of mv in the two DR planes (host-
    prepared, mv numerically exact to ~0.2%), rhs is the fp8 exp tile
    broadcast (stride-0) into both planes.  Only the exp-side e4m3
    rounding of those tiles adds error; measured rel err 1.77e-2 vs the
    2e-2 gate (42 tiles measured 1.82e-2 -- the slope is super-linear;
    41 keeps ~11% gate headroom).  (An error-free 3-product residual variant exists behind
    the pattern table -- 'R' tiles -- but its DVE hi/lo-split latency
    stalls the PE more than the saved cycles; left dormant.)
  * All readouts are emitted with a uniform two-iteration lag behind
    their tile's score/exp so cross-engine chain latency never stalls
    the PE; scores run two tiles ahead (three psum score banks).
  * THW zero-padded to 13056 = 102*128; pad tokens carry a poison value
    in the norm channel driving logits to -80 (exp -> 0).
  * mv is DMA'd ONCE into resident SBUF (bf16 tiles + fp8 hi/lo pairs,
    1KiB/partition/tile either way) and reused by both query blocks.
  * Scores are software-pipelined two m-tiles ahead of the readout.
  * Startup: the first mkq chunks ride the ACT and DVE DGE queues so
    they don't serialize behind the SP queue (650ns per issue); mv8
    rides the Pool SWDGE queue.  Junk matmuls on DVE-memset SBUF keep
    the PE busy (and burn the p-state ramp) exactly until the head DMA
    lands.
  * Query blocks (512, 298): psum-bank-sized; the small block last
    shortens the endgame.  Outputs are processed as bank-PAIRS: the acc
    psum is allocated as two [P, 2, 512] tiles, normalized by two
    grouped DVE muls against the PE-broadcast recip row, and leave as
    two 2-row-per-partition DMAs on different DGE queues.
  * Denominator in fp16 (2x DVE mode, ~8x less rounding than bf16);
    fp8 tiles accumulate via mixed fp8+fp16 DVE adds; the last tile's
    exp + den-tail + recip + bcast are hoisted a tile early so the
    normalization chain overlaps the final readouts; block 0's output
    is evacuated to fp16 at the boundary and normalized later in block
    1's DVE slack.
"""

import os
import sys

import numpy as np
import ml_dtypes

for _p in ("/opt/trn_rl_repo",):
    if _p not in sys.path and os.path.isdir(_p):
        sys.path.insert(0, _p)

B, CK, T, H, W = 4, 64, 8, 30, 54
CV = 512
THW = T * H * W          # 12960
NQ = H * W               # 1620
QH = NQ // 2             # 810   per-core query half
QBLKS = [(0, 512), (512, 298)]  # two query passes (psum slice padded to 512)
P = 128
NT = 102                 # padded m-tiles
THWP = NT * P            # 13056
# Tile kinds over the 78 mid tiles [20, 98): N = fp8 DoubleRow with e4m3
# exp (cheap, noisy); R = fp8 DoubleRow 3-product residual pairs (cheap,
# accurate, needs DVE prep); B = bf16.  Edge tiles stay bf16.
_P13 = "RRNNBNRRNNBNN"   # R4 N7 B2
_P13c = "RRNBNBRRNBNNB"  # R4 N5 B4
_PATTERN = _P13 + _P13 + _P13c + _P13 + _P13 + _P13c
if 1:  # KR0 experiment: 38 N spread evenly, no residual tiles
    _pos = sorted(int((i + 0.5) * 78 / 41) for i in range(41))
    _PATTERN = "".join("N" if p in _pos else "B" for p in range(78))
assert len(_PATTERN) == 78
KIND_BY_TILE = ["B"] * 20 + list(_PATTERN) + ["B"] * 4
K8 = KIND_BY_TILE.count("N")   # 38 noisy fp8 tiles
KR = KIND_BY_TILE.count("R")   # 24 residual fp8 tiles (12 pairs)
NB = KIND_BY_TILE.count("B")   # 40 bf16 m-tiles
N8SLOT = 2 * K8 + 5 * (KR // 2)  # mv8 slots: (hi,lo) / (hiA,loA,hiA,hiB,loB)
PAD_POISON = 640.0       # pad-token norm channel: psum=-320 -> logit=-80
EXP_BIAS = 1.5           # exp(0.25x + 1.5); e^1.5 cancels in normalization
CH = 3                   # mkq m-tiles per DMA chunk (102 = 34*3)
NCHUNK = NT // CH        # 34
JUNK_128 = 5             # PE-warmup junk matmuls of 128 cols (early)
JUNK_512 = 5             # PE-warmup junk matmuls of NQ0 cols

# tile index -> storage slot.  B: index into mvp; N: base of its (hi,lo)
# pair in mv8p; R (first of pair): base of the 5-slot group; R (second):
# base + 3 (its (hi,lo) sit at +3,+4; the cross instr reads +2,+3).
_SLOT = [0] * NT
_RFIRST = [False] * NT
_nb = _n8 = 0
_pend = None
for _t in range(NT):
    k = KIND_BY_TILE[_t]
    if k == "B":
        _SLOT[_t] = _nb
        _nb += 1
    elif k == "N":
        _SLOT[_t] = _n8
        _n8 += 2
    else:
        if _pend is None:
            _pend = _t
            _RFIRST[_t] = True
            _SLOT[_t] = _n8
        else:
            _SLOT[_t] = _n8 + 3
            _n8 += 5
            _pend = None
assert _pend is None and _nb == NB and _n8 == N8SLOT

_PROGRAM = None
_BF16 = ml_dtypes.bfloat16
_FP8 = ml_dtypes.float8_e4m3fn


def _build_program():
    import concourse.mybir as mybir
    import concourse.tile as tile
    from concourse import bacc

    f32 = mybir.dt.float32
    f32r = mybir.dt.float32r
    bf16 = mybir.dt.bfloat16
    fp16 = mybir.dt.float16
    fp8 = mybir.dt.float8e4
    Exp = mybir.ActivationFunctionType.Exp
    DR = mybir.MatmulPerfMode.DoubleRow

    nc = bacc.Bacc(
        "TRN2",
        target_bir_lowering=False,
        debug=False,
        enable_asserts=False,
        num_devices=8,
    )

    mkq = nc.dram_tensor("mkq", [P, THWP], bf16, kind="ExternalInput").ap()
    qkc = nc.dram_tensor("qkc", [P, QH], bf16, kind="ExternalInput").ap()
    NQ0 = QBLKS[0][1]
    PSW = 512             # bank-aligned pair-slice width
    HEADC = NQ0 + CH * P  # qkc block-0 + mkq chunk 0
    head = nc.dram_tensor("head", [P, HEADC], bf16, kind="ExternalInput").ap()
    mvp = nc.dram_tensor("mvp", [P, NB, CV], bf16, kind="ExternalInput").ap()
    mv8p = nc.dram_tensor("mv8p", [P, N8SLOT, CV], fp8, kind="ExternalInput").ap()
    out = nc.dram_tensor("out", [CV, QH], bf16, kind="ExternalOutput").ap()

    NVT = 2 * NT  # virtual tiles: (block, m-tile) flattened

    with tile.TileContext(nc) as tc:
        with (
            tc.tile_pool(name="const", bufs=1) as cpool,
            tc.tile_pool(name="exp", bufs=10) as expool,
            tc.tile_pool(name="exp8", bufs=8) as ex8pool,
            tc.tile_pool(name="den", bufs=2) as dpool,
            tc.tile_pool(name="vec", bufs=2) as vpool,
            tc.tile_pool(name="outp", bufs=4) as opool,
            tc.tile_pool(name="wlo", bufs=3) as wlopool,
            tc.tile_pool(name="score_ps", bufs=3, space="PSUM") as spspool,
            tc.tile_pool(name="acc_ps", bufs=1, space="PSUM") as apspool,
            tc.tile_pool(name="den_ps", bufs=1, space="PSUM") as dpspool,
        ):
            # PE warm-up: junk matmuls on DVE-memset SBUF burn the p-state
            # ramp until the head DMA lands.  The first few use the small
            # junk_w tile as both operands so they can start the moment its
            # memset finishes.
            junk_w = cpool.tile([P, P], bf16, tag="junk_w", name="junk_w")
            nc.gpsimd.memset(junk_w[:], 0.0)
            junk_r = cpool.tile([P, NQ0], bf16, tag="junk_r", name="junk_r")
            # on the Pool queue: also delays mv01's SWDGE gen just enough
            # that mkq1 wins the serial-DMA slot after the head transfer.
            nc.gpsimd.memset(junk_r[:], 0.0)
            for _ in range(JUNK_128):
                jp = spspool.tile([P, NQ0], f32, tag="score", name="warm")
                nc.tensor.matmul(
                    jp[:, :P], lhsT=junk_w[:], rhs=junk_w[:], start=True, stop=True
                )
            for _ in range(JUNK_512):
                jp = spspool.tile([P, NQ0], f32, tag="score", name="warm")
                nc.tensor.matmul(
                    jp[:], lhsT=junk_w[:], rhs=junk_r[:], start=True, stop=True
                )

            head_sb = cpool.tile([P, HEADC], bf16, tag="head", name="head")
            mkq_sb = cpool.tile([P, THWP - CH * P], bf16, tag="mkq", name="mkq")
            qkc_sb = cpool.tile([P, QH - NQ0], bf16, tag="qkc", name="qkc")
            mv_sb = cpool.tile([P, NB * CV], bf16, tag="mv", name="mv")
            mv8_sb = cpool.tile([P, N8SLOT, CV], fp8, tag="mv8", name="mv8")
            # Whole head (block-0 queries + mkq tiles 0-2) in one SP DMA;
            # mkq chunks 1-2 ride the ACT/DVE queues so they don't serialize
            # behind SP's 650ns-per-issue; mv tiles 0-1 + mv8 ride Pool SWDGE.
            # Need-ordered DMA stream.  The cost model serializes transfers on
            # one DMA resource, so order = priority: head (tiles 0-2 +
            # queries), then mkq chunks strictly ahead of the mv stream;
            # mv tiles 0-1 ride Pool SWDGE (its descriptor gen overlaps the
            # head transfer); mv8 goes in quarters so it never blocks mkq.
            nc.sync.dma_start(out=head_sb[:], in_=head[:])
            nc.sync.dma_start(
                out=mkq_sb[:, : CH * P], in_=mkq[:, CH * P : 2 * CH * P]
            )
            nc.sync.dma_start(out=mv_sb[:, : 2 * CV], in_=mvp[:, :2, :])
            nc.sync.dma_start(
                out=mkq_sb[:, CH * P : 2 * CH * P], in_=mkq[:, 2 * CH * P : 3 * CH * P]
            )
            Q8 = (N8SLOT + 7) // 8  # mv8 eighth (in slots)
            vnext = 2
            for i in range(3, NCHUNK):
                k0, k1 = i * CH * P, (i + 1) * CH * P
                nc.sync.dma_start(
                    out=mkq_sb[:, k0 - CH * P : k1 - CH * P], in_=mkq[:, k0:k1]
                )
                if i == 4:
                    q1, nq1 = QBLKS[1]
                    nc.sync.dma_start(out=qkc_sb[:], in_=qkc[:, q1 : q1 + nq1])
                if i in (6, 9, 12, 15, 18, 21, 24, 27):
                    q = (6, 9, 12, 15, 18, 21, 24, 27).index(i)
                    s0 = Q8 * q
                    s1 = Q8 * (q + 1) if q < 7 else N8SLOT
                    nc.sync.dma_start(
                        out=mv8_sb[:, s0:s1, :], in_=mv8p[:, s0:s1, :]
                    )
                v1 = min(vnext + 2, NB)
                if vnext < v1:
                    nc.sync.dma_start(
                        out=mv_sb[:, vnext * CV : v1 * CV], in_=mvp[:, vnext:v1, :]
                    )
                    vnext = v1
            if vnext < NB:
                nc.sync.dma_start(
                    out=mv_sb[:, vnext * CV :], in_=mvp[:, vnext:, :]
                )

            bias_sb = cpool.tile([P, 1], f32, tag="bias", name="bias")
            nc.vector.memset(bias_sb[:], EXP_BIAS)
            ones_col_f = cpool.tile([P, 1], f32, tag="ones_col_f", name="ones_col_f")
            nc.vector.memset(ones_col_f[:], 1.0)
            ones_col = cpool.tile([P, 1], f32r, tag="ones_col", name="ones_col")
            with nc.allow_low_precision("exact 1.0 cast to f32r"):
                nc.vector.tensor_copy(ones_col[:], ones_col_f[:])
            ones_col_b = cpool.tile([P, 1], bf16, tag="ones_col_b", name="ones_col_b")
            nc.vector.memset(ones_col_b[:], 1.0)
            ones_col_h = cpool.tile([P, 1], fp16, tag="ones_col_h", name="ones_col_h")
            nc.vector.memset(ones_col_h[:], 1.0)
            ones_row_f = cpool.tile([1, P], f32, tag="ones_row_f", name="ones_row_f")
            nc.vector.memset(ones_row_f[:], 1.0)
            ones_row = cpool.tile([1, P], f32r, tag="ones_row", name="ones_row")
            with nc.allow_low_precision("exact 1.0 cast to f32r"):
                nc.vector.tensor_copy(ones_row[:], ones_row_f[:])

            # Per-block state created lazily at block entry.
            rstate = [None]  # current residual pair's w_lo buffer
            accs = [None, None]   # each: two [P, 2, 512] psum bank-pairs
            dens = [None, None]
            scores = [None] * NVT

            def vt_block(j):
                return 0 if j < NT else 1

            def emit_score(j):
                blk = vt_block(j)
                q0, nq = QBLKS[blk]
                mi = j - blk * NT
                if mi < CH:
                    lhsT = head_sb[:, NQ0 + mi * P : NQ0 + (mi + 1) * P]
                else:
                    lhsT = mkq_sb[:, (mi - CH) * P : (mi - CH + 1) * P]
                rhs = head_sb[:, :NQ0] if blk == 0 else qkc_sb[:]
                s = spspool.tile([P, nq], f32, tag="score", name="score")
                nc.tensor.matmul(s[:], lhsT=lhsT, rhs=rhs, start=True, stop=True)
                scores[j] = s

            def emit_block_entry_dens(blk):
                d = dpool.tile([P, QBLKS[blk][1]], fp16, tag="den", name="den")
                nc.vector.memset(d[:], 0.0)
                dens[blk] = d

            def emit_block_entry_accs(blk):
                accs[blk] = [
                    apspool.tile([P, 2, PSW], f32, tag=f"accp{p}", name=f"accp{p}")
                    for p in range(2)
                ]

            recips = [None, None]
            den_sums = [None, None]

            def emit_den_main(blk):
                # Partial denominator (tiles 0..NT-2) reduced on PE right
                # after the second-to-last DVE add -- off the endgame chain.
                nq = QBLKS[blk][1]
                den_sum = dpspool.tile([1, nq], f32, tag="den_sum", name="den_sum")
                nc.tensor.matmul(
                    den_sum[:],
                    lhsT=ones_col_h[:],
                    rhs=dens[blk][:],
                    start=True,
                    stop=False,
                )
                den_sums[blk] = den_sum

            def emit_den_tail_recip(blk, ex_last):
                # Last tile's exp summed straight off ACT's output on the PE,
                # skipping the last DVE accumulator add on the endgame chain.
                nq = QBLKS[blk][1]
                nc.tensor.matmul(
                    den_sums[blk][:],
                    lhsT=ones_col_b[:],
                    rhs=ex_last[:],
                    start=False,
                    stop=True,
                )
                r = vpool.tile([1, nq], f32r, tag="recip", name="recip")
                with nc.allow_low_precision("feeds f32r broadcast matmul"):
                    nc.vector.reciprocal(r[:], den_sums[blk][:])
                recips[blk] = r

            evacs = [None, None]

            def emit_evac(blk):
                # ACT evacuates the two acc bank-pairs to fp16 SBUF, freeing
                # the psum banks for the next block ~2.5us sooner than
                # normalize-in-place; normalization happens later during the
                # next block's DVE slack.
                q0, nq = QBLKS[blk]
                evacs[blk] = []
                for pair in range(2):
                    e = cpool.tile([P, 2, PSW], fp16, tag=f"evac{pair}", name=f"evac{pair}")
                    nc.scalar.copy(e[:, :, :nq], accs[blk][pair][:, :, :nq])
                    evacs[blk].append(e)

            def emit_norm_deferred(blk):
                q0, nq = QBLKS[blk]
                bcast_ps = spspool.tile([P, nq], f32, tag="score", name="bcast")
                nc.tensor.matmul(
                    bcast_ps[:],
                    lhsT=ones_row[:],
                    rhs=recips[blk][:],
                    start=True,
                    stop=True,
                )
                bcast_sb = vpool.tile([P, nq], bf16, tag="bcast_sb", name="bcast_sb")
                nc.vector.tensor_copy(bcast_sb[:], bcast_ps[:])
                rep = bcast_sb[:].unsqueeze(1).broadcast_to([P, 2, nq])
                engs = [nc.sync, nc.gpsimd]
                for pair in range(2):
                    o = opool.tile([P, 2, PSW], bf16, tag="out", name="out")
                    nc.vector.tensor_mul(
                        o[:, :, :nq], evacs[blk][pair][:, :, :nq], rep
                    )
                    dst = out[
                        pair * 2 * P : (pair + 1) * 2 * P, q0 : q0 + nq
                    ].rearrange("(two p) q -> p two q", two=2)
                    engs[pair].dma_start(out=dst, in_=o[:, :, :nq])

            def emit_bcast(blk):
                # bcast the per-query recip onto 128 partitions (PE), copy it
                # to SBUF (ACT).  Emitted BEFORE the last tile's readouts so
                # this chain overlaps them.
                q0, nq = QBLKS[blk]
                bcast_ps = spspool.tile([P, nq], f32, tag="score", name="bcast")
                nc.tensor.matmul(
                    bcast_ps[:],
                    lhsT=ones_row[:],
                    rhs=recips[blk][:],
                    start=True,
                    stop=True,
                )
                bcast_sb = vpool.tile([P, nq], bf16, tag="bcast_sb", name="bcast_sb")
                nc.scalar.copy(bcast_sb[:], bcast_ps[:])
                return bcast_sb

            def emit_muls(blk, bcast_sb):
                # Normalize each acc bank-PAIR in one grouped DVE mul and
                # ship it as one 2-row-per-partition DMA.
                q0, nq = QBLKS[blk]
                rep = bcast_sb[:].unsqueeze(1).broadcast_to([P, 2, nq])
                engs = [nc.sync, nc.sync]
                for pair in range(2):
                    o = opool.tile([P, 2, PSW], bf16, tag="out", name="out")
                    nc.vector.tensor_mul(
                        o[:, :, :nq], accs[blk][pair][:, :, :nq], rep
                    )
                    dst = out[
                        pair * 2 * P : (pair + 1) * 2 * P, q0 : q0 + nq
                    ].rearrange("(two p) q -> p two q", two=2)
                    engs[pair].dma_start(out=dst, in_=o[:, :, :nq])

            def _dr4(blk, nq, base, rhs, start, stop):
                for c in range(4):
                    nc.tensor.matmul(
                        accs[blk][c // 2][:, c % 2, :nq],
                        lhsT=mv8_sb[:, base : base + 2, c * P : (c + 1) * P],
                        rhs=rhs,
                        start=start,
                        stop=stop,
                        perf_mode=DR,
                    )

            def emit_readout(j, ex, whi=None, wlo=None):
                blk = vt_block(j)
                mi = j - blk * NT
                nq = QBLKS[blk][1]
                kind = KIND_BY_TILE[mi]
                sl = _SLOT[mi]
                if kind == "B":
                    def _ro(blk=blk, nq=nq, sl=sl, mi=mi, ex=ex):
                        for c in range(4):
                            nc.tensor.matmul(
                                accs[blk][c // 2][:, c % 2, :nq],
                                lhsT=mv_sb[
                                    :, sl * CV + c * P : sl * CV + (c + 1) * P
                                ],
                                rhs=ex[:],
                                start=(mi == 0),
                                stop=(mi == NT - 1),
                            )
                elif kind == "N":
                    def _ro(blk=blk, nq=nq, sl=sl, mi=mi, ex=ex):
                        rhs = ex[:].unsqueeze(1).broadcast_to([P, 2, nq])
                        _dr4(blk, nq, sl, rhs, mi == 0, mi == NT - 1)
                else:
                    def _ro(blk=blk, nq=nq, sl=sl, whi=whi):
                        rhs = whi[:].unsqueeze(1).broadcast_to([P, 2, nq])
                        _dr4(blk, nq, sl, rhs, False, False)
                    if not _RFIRST[mi]:
                        # second of pair: the cross instr (hiA, hiB) x
                        # (w_lo_a, w_lo_b) waits on the DVE lo-split; give it
                        # one more iteration of slack.
                        def _rx(blk=blk, nq=nq, sl=sl, wlo=wlo):
                            _dr4(blk, nq, sl - 1, wlo[:, :, :nq], False, False)
                        pending.append((j + 3, _rx))
                pending.append((j + 2, _ro))

            emit_block_entry_dens(0)
            emit_block_entry_accs(0)
            emit_score(0)
            emit_score(1)
            pending = []
            for j in range(NVT):
                if j == NT:
                    for jj in range(j + 2, j + 6):
                        emit_score(jj)
                elif j + 2 < NVT and not (NT < j < NT + 4):
                    emit_score(j + 2)
                if j == NT + 1:
                    # block 1's acc psum rotates in only now: every access to
                    # block 0's acc tiles has been emitted.
                    emit_block_entry_accs(1)
                for item in pending[:]:
                    if item[0] <= j:
                        item[1]()
                        pending.remove(item)
                blk = vt_block(j)
                if j == NT:
                    emit_block_entry_dens(1)
                q0, nq = QBLKS[blk]
                mi = j - blk * NT
                if mi == NT - 1 and j == NVT - 1:
                    # exp/den-tail/recip were hoisted into the previous
                    # iteration: flush r/o[NVT-2], then bcast, then the last
                    # readouts inline so the bcast chain overlaps them.
                    for item in pending:
                        item[1]()
                    pending.clear()
                    final_bcast = emit_bcast(1)
                    for c in range(4):
                        nc.tensor.matmul(
                            accs[blk][c // 2][:, c % 2, :nq],
                            lhsT=mv_sb[
                                :,
                                _SLOT[mi] * CV + c * P : _SLOT[mi] * CV
                                + (c + 1) * P,
                            ],
                            rhs=hoisted_ex[:],
                            start=False,
                            stop=True,
                        )
                    continue
                kind = KIND_BY_TILE[mi]
                ex = (
                    ex8pool.tile([P, nq], fp8, tag="exp8", name="exp8")
                    if kind == "N"
                    else expool.tile([P, nq], bf16, tag="exp", name="exp")
                )
                nc.scalar.activation(
                    ex[:], scores[j][:], Exp, bias=bias_sb[:], scale=0.25
                )
                scores[j] = None
                whi = wlo = None
                if kind == "R":
                    whi = ex8pool.tile([P, nq], fp8, tag="exp8", name="whi")
                    if _RFIRST[mi]:
                        rstate[0] = wlopool.tile(
                            [P, 2, PSW], fp8, tag="wlo", name="wlo"
                        )
                    wlo = rstate[0]
                    plane = 0 if _RFIRST[mi] else 1
                    with nc.allow_low_precision("fp8 exp hi/lo split"):
                        nc.vector.tensor_copy(whi[:], ex[:])
                        nc.vector.scalar_tensor_tensor(
                            wlo[:, plane, :nq],
                            ex[:],
                            1.0,
                            whi[:],
                            mybir.AluOpType.mult,
                            mybir.AluOpType.subtract,
                        )
                if mi < NT - 1:
                    with nc.allow_low_precision("den feeds f32r den_sum matmul"):
                        nc.vector.tensor_add(dens[blk][:], dens[blk][:], ex[:])
                    if mi == NT - 2:
                        emit_den_main(blk)
                        if j == NVT - 2:
                            # Hoist the final tile's exp + den tail + recip so
                            # the bcast chain overlaps the last 8 readouts.
                            hoisted_ex = expool.tile(
                                [P, nq], bf16, tag="exp", name="exp"
                            )
                            nc.scalar.activation(
                                hoisted_ex[:],
                                scores[NVT - 1][:],
                                Exp,
                                bias=bias_sb[:],
                                scale=0.25,
                            )
                            scores[NVT - 1] = None
                            emit_den_tail_recip(blk, hoisted_ex)
                else:
                    emit_den_tail_recip(blk, ex)
                if j == NT:
                    # Block 1's outputs: flush block 0's remaining readouts,
                    # evacuate (fast bank free), normalize later during block
                    # 1's slack.
                    for item in pending:
                        item[1]()
                    pending.clear()
                    emit_evac(0)
                if j == NT + 8:
                    emit_norm_deferred(0)
                emit_readout(j, ex, whi, wlo)
            emit_muls(1, final_bcast)

    nc.compile()
    return nc


def _get_program():
    global _PROGRAM
    if _PROGRAM is None:
        _PROGRAM = _build_program()
    return _PROGRAM


def _make_in_maps(mk, qk, mv):
    mkf = np.asarray(mk, dtype=np.float32).reshape(B, CK, THW)
    qkf = np.asarray(qk, dtype=np.float32).reshape(B, CK, NQ)
    mvf = np.asarray(mv, dtype=np.float32).reshape(B, CV, THW)

    bf_idx = [t for t in range(NT) if KIND_BY_TILE[t] == "B"]

    in_maps = []
    for b in range(B):
        mkq_b = np.zeros((P, THWP), dtype=_BF16)
        mkq_b[:CK, :THW] = mkf[b]
        mkq_b[CK:, :THW] = mkf[b] * mkf[b]
        mkq_b[CK, THW:] = PAD_POISON  # pad tokens -> logit -80 -> exp ~ 0

        mvt = np.zeros((THWP, CV), dtype=np.float32)
        mvt[:THW] = mvf[b].T
        mvp_full = np.ascontiguousarray(mvt.reshape(NT, P, CV).transpose(1, 0, 2))
        mvp_b = mvp_full[:, bf_idx, :].astype(_BF16)
        hi_all = mvp_full.astype(_FP8)
        lo_all = (mvp_full - hi_all.astype(np.float32)).astype(_FP8)
        mv8_b = np.zeros((P, N8SLOT, CV), dtype=_FP8)
        for t in range(NT):
            k = KIND_BY_TILE[t]
            s = _SLOT[t]
            if k == "N":
                mv8_b[:, s] = hi_all[:, t]
                mv8_b[:, s + 1] = lo_all[:, t]
            elif k == "R" and _RFIRST[t]:
                mv8_b[:, s] = hi_all[:, t]      # hiA
                mv8_b[:, s + 1] = lo_all[:, t]  # loA
                mv8_b[:, s + 2] = hi_all[:, t]  # hiA dup (cross instr)
            elif k == "R":
                mv8_b[:, s] = hi_all[:, t]      # hiB (slot base+3)
                mv8_b[:, s + 1] = lo_all[:, t]  # loB

        for h in range(2):
            qkc_b = np.empty((P, QH), dtype=_BF16)
            qkc_b[:CK] = qkf[b][:, h * QH : (h + 1) * QH]
            qkc_b[CK:] = -0.5
            nq0 = QBLKS[0][1]
            head_b = np.concatenate([qkc_b[:, :nq0], mkq_b[:, : CH * P]], axis=1)
            in_maps.append(
                {
                    "mkq": mkq_b,
                    "qkc": qkc_b,
                    "mvp": mvp_b,
                    "mv8p": mv8_b,
                    "head": head_b,
                }
            )
    return in_maps


def kernel(mk, qk, mv, _trace=False, _results_out=None):
    from concourse import bass_utils

    nc = _get_program()
    in_maps = _make_in_maps(mk, qk, mv)
    res = bass_utils.run_bass_kernel_spmd(
        nc, in_maps, core_ids=list(range(8)), trace=_trace
    )
    if _results_out is not None:
        _results_out.append(res)

    full = np.empty((B, CV, NQ), dtype=np.float32)
    for b in range(B):
        for h in range(2):
            full[b][:, h * QH : (h + 1) * QH] = np.asarray(
                res.results[2 * b + h]["out"], dtype=np.float32
            )
    return full.reshape(B, CV, H, W)


# revision 49
# speedup vs baseline: 1.0054x; 1.0003x over previous
"""Trainium2 Bass kernel for the MemoryReader (retrieval-knn) module.

Math (per batch b):
    a[m]     = sum_ck mk[ck, m]^2
    logits   = (2 * mk^T qk - a) / sqrt(CK)        # [THW, NQ]
    aff      = softmax(logits, axis=THW)
    out      = mv @ aff                            # [CV, NQ]

Shapes: B=4, CK=64, T=8, H=30, W=54 (THW=12960, NQ=1620), CV=512.

Sharding: 8 cores = (B=4) x (NQ halves of 810).  Softmax is over THW,
which every core owns fully, so no cross-core reduction is needed.

The squared-norm term is folded into the score matmul by augmenting the
contraction dim to K=128:
    lhsT' = [mk ; mk^2]  (host-prepared, [128, THW])
    rhs'  = [qk ; -0.5 ]  (host-prepared, [128, 810])
    psum  = mk.qk - a/2  ->  logits = 0.25 * psum  (ACT scale)
All exps carry a global bias EXP_BIAS (exp(0.25 x + 1.5)); the factor
e^1.5 cancels in the final normalization and pushes the fp8 exp tiles
(below) up into e4m3's well-covered range.

Performance structure (per core, cost-model-driven; ~155.5us vs the
183.2us all-bf16 predecessor):
  * Matmul operands bf16 except: 41 of the 102 m-tiles (evenly spread
    over [20, 98) so ACT's exp keeps pace and block edges stay bf16) run
    the readout in fp8e4 DoubleRow mode (0.5 cycles/col): lhsT holds
    (mv_hi, mv_lo) e4m3 splits of mv in the two DR planes (host-
    prepared, mv numerically exact to ~0.2%), rhs is the fp8 exp tile
    broadcast (stride-0) into both planes.  Only the exp-side e4m3
    rounding of those tiles adds error; measured rel err 1.77e-2 vs the
    2e-2 gate (42 tiles measured 1.82e-2 -- the slope is super-linear;
    41 keeps ~11% gate headroom).  (An error-free 3-product residual variant exists behind
    the pattern table -- 'R' tiles -- but its DVE hi/lo-split latency
    stalls the PE more than the saved cycles; left dormant.)
  * All readouts are emitted with a uniform two-iteration lag behind
    their tile's score/exp so cross-engine chain latency never stalls
    the PE; scores run two tiles ahead (three psum score banks).
  * THW zero-padded to 13056 = 102*128; pad tokens carry a poison value
    in the norm channel driving logits to -80 (exp -> 0).
  * mv is DMA'd ONCE into resident SBUF (bf16 tiles + fp8 hi/lo pairs,
    1KiB/partition/tile either way) and reused by both query blocks.
  * Scores are software-pipelined two m-tiles ahead of the readout.
  * Startup: the first mkq chunks ride the ACT and DVE DGE queues so
    they don't serialize behind the SP queue (650ns per issue); mv8
    rides the Pool SWDGE queue.  Junk matmuls on DVE-memset SBUF keep
    the PE busy (and burn the p-state ramp) exactly until the head DMA
    lands.
  * Query blocks (512, 298): psum-bank-sized; the small block last
    shortens the endgame.  Outputs are processed as bank-PAIRS: the acc
    psum is allocated as two [P, 2, 512] tiles, normalized by two
    grouped DVE muls against the PE-broadcast recip row, and leave as
    two 2-row-per-partition DMAs on different DGE queues.
  * Denominator in fp16 (2x DVE mode, ~8x less rounding than bf16);
    fp8 tiles accumulate via mixed fp8+fp16 DVE adds; the last tile's
    exp + den-tail + recip + bcast are hoisted a tile early so the
    normalization chain overlaps the final readouts; block 0's output
    is evacuated to fp16 at the boundary and normalized later in block
    1's DVE slack.
"""

import os
import sys

import numpy as np
import ml_dtypes

for _p in ("/opt/trn_rl_repo",):
    if _p not in sys.path and os.path.isdir(_p):
        sys.path.insert(0, _p)

B, CK, T, H, W = 4, 64, 8, 30, 54
CV = 512
THW = T * H * W          # 12960
NQ = H * W               # 1620
QH = NQ // 2             # 810   per-core query half
QBLKS = [(0, 512), (512, 298)]  # two query passes (psum slice padded to 512)
P = 128
NT = 102                 # padded m-tiles
THWP = NT * P            # 13056
# Tile kinds over the 78 mid tiles [20, 98): N = fp8 DoubleRow with e4m3
# exp (cheap, noisy); R = fp8 DoubleRow 3-product residual pairs (cheap,
# accurate, needs DVE prep); B = bf16.  Edge tiles stay bf16.
_P13 = "RRNNBNRRNNBNN"   # R4 N7 B2
_P13c = "RRNBNBRRNBNNB"  # R4 N5 B4
_PATTERN = _P13 + _P13 + _P13c + _P13 + _P13 + _P13c
if 1:  # KR0 experiment: 38 N spread evenly, no residual tiles
    _pos = sorted(int((i + 0.5) * 78 / 41) for i in range(41))
    _PATTERN = "".join("N" if p in _pos else "B" for p in range(78))
assert len(_PATTERN) == 78
KIND_BY_TILE = ["B"] * 20 + list(_PATTERN) + ["B"] * 4
K8 = KIND_BY_TILE.count("N")   # 38 noisy fp8 tiles
KR = KIND_BY_TILE.count("R")   # 24 residual fp8 tiles (12 pairs)
NB = KIND_BY_TILE.count("B")   # 40 bf16 m-tiles
N8SLOT = 2 * K8 + 5 * (KR // 2)  # mv8 slots: (hi,lo) / (hiA,loA,hiA,hiB,loB)
PAD_POISON = 640.0       # pad-token norm channel: psum=-320 -> logit=-80
EXP_BIAS = 1.5           # exp(0.25x + 1.5); e^1.5 cancels in normalization
CH = 3                   # mkq m-tiles per DMA chunk (102 = 34*3)
NCHUNK = NT // CH        # 34
JUNK_128 = 5             # PE-warmup junk matmuls of 128 cols (early)
JUNK_512 = 5             # PE-warmup junk matmuls of NQ0 cols

# tile index -> storage slot.  B: index into mvp; N: base of its (hi,lo)
# pair in mv8p; R (first of pair): base of the 5-slot group; R (second):
# base + 3 (its (hi,lo) sit at +3,+4; the cross instr reads +2,+3).
_SLOT = [0] * NT
_RFIRST = [False] * NT
_nb = _n8 = 0
_pend = None
for _t in range(NT):
    k = KIND_BY_TILE[_t]
    if k == "B":
        _SLOT[_t] = _nb
        _nb += 1
    elif k == "N":
        _SLOT[_t] = _n8
        _n8 += 2
    else:
        if _pend is None:
            _pend = _t
            _RFIRST[_t] = True
            _SLOT[_t] = _n8
        else:
            _SLOT[_t] = _n8 + 3
            _n8 += 5
            _pend = None
assert _pend is None and _nb == NB and _n8 == N8SLOT

_PROGRAM = None
_BF16 = ml_dtypes.bfloat16
_FP8 = ml_dtypes.float8_e4m3fn


def _build_program():
    import concourse.mybir as mybir
    import concourse.tile as tile
    from concourse import bacc

    f32 = mybir.dt.float32
    f32r = mybir.dt.float32r
    bf16 = mybir.dt.bfloat16
    fp16 = mybir.dt.float16
    fp8 = mybir.dt.float8e4
    Exp = mybir.ActivationFunctionType.Exp
    DR = mybir.MatmulPerfMode.DoubleRow

    nc = bacc.Bacc(
        "TRN2",
        target_bir_lowering=False,
        debug=False,
        enable_asserts=False,
        num_devices=8,
    )

    mkq = nc.dram_tensor("mkq", [P, THWP], bf16, kind="ExternalInput").ap()
    qkc = nc.dram_tensor("qkc", [P, QH], bf16, kind="ExternalInput").ap()
    NQ0 = QBLKS[0][1]
    PSW = 512             # bank-aligned pair-slice width
    HEADC = NQ0 + CH * P  # qkc block-0 + mkq chunk 0
    head = nc.dram_tensor("head", [P, HEADC], bf16, kind="ExternalInput").ap()
    mvp = nc.dram_tensor("mvp", [P, NB, CV], bf16, kind="ExternalInput").ap()
    mv8p = nc.dram_tensor("mv8p", [P, N8SLOT, CV], fp8, kind="ExternalInput").ap()
    out = nc.dram_tensor("out", [CV, QH], bf16, kind="ExternalOutput").ap()

    NVT = 2 * NT  # virtual tiles: (block, m-tile) flattened

    with tile.TileContext(nc) as tc:
        with (
            tc.tile_pool(name="const", bufs=1) as cpool,
            tc.tile_pool(name="exp", bufs=10) as expool,
            tc.tile_pool(name="exp8", bufs=8) as ex8pool,
            tc.tile_pool(name="den", bufs=2) as dpool,
            tc.tile_pool(name="vec", bufs=2) as vpool,
            tc.tile_pool(name="outp", bufs=4) as opool,
            tc.tile_pool(name="wlo", bufs=3) as wlopool,
            tc.tile_pool(name="score_ps", bufs=3, space="PSUM") as spspool,
            tc.tile_pool(name="acc_ps", bufs=1, space="PSUM") as apspool,
            tc.tile_pool(name="den_ps", bufs=1, space="PSUM") as dpspool,
        ):
            # PE warm-up: junk matmuls on DVE-memset SBUF burn the p-state
            # ramp until the head DMA lands.  The first few use the small
            # junk_w tile as both operands so they can start the moment its
            # memset finishes.
            junk_w = cpool.tile([P, P], bf16, tag="junk_w", name="junk_w")
            nc.gpsimd.memset(junk_w[:], 0.0)
            junk_r = cpool.tile([P, NQ0], bf16, tag="junk_r", name="junk_r")
            # on the Pool queue: also delays mv01's SWDGE gen just enough
            # that mkq1 wins the serial-DMA slot after the head transfer.
            nc.gpsimd.memset(junk_r[:], 0.0)
            for _ in range(JUNK_128):
                jp = spspool.tile([P, NQ0], f32, tag="score", name="warm")
                nc.tensor.matmul(
                    jp[:, :P], lhsT=junk_w[:], rhs=junk_w[:], start=True, stop=True
                )
            for _ in range(JUNK_512):
                jp = spspool.tile([P, NQ0], f32, tag="score", name="warm")
                nc.tensor.matmul(
                    jp[:], lhsT=junk_w[:], rhs=junk_r[:], start=True, stop=True
                )

            head_sb = cpool.tile([P, HEADC], bf16, tag="head", name="head")
            mkq_sb = cpool.tile([P, THWP - CH * P], bf16, tag="mkq", name="mkq")
            qkc_sb = cpool.tile([P, QH - NQ0], bf16, tag="qkc", name="qkc")
            mv_sb = cpool.tile([P, NB * CV], bf16, tag="mv", name="mv")
            mv8_sb = cpool.tile([P, N8SLOT, CV], fp8, tag="mv8", name="mv8")
            # Whole head (block-0 queries + mkq tiles 0-2) in one SP DMA;
            # mkq chunks 1-2 ride the ACT/DVE queues so they don't serialize
            # behind SP's 650ns-per-issue; mv tiles 0-1 + mv8 ride Pool SWDGE.
            # Need-ordered DMA stream.  The cost model serializes transfers on
            # one DMA resource, so order = priority: head (tiles 0-2 +
            # queries), then mkq chunks strictly ahead of the mv stream;
            # mv tiles 0-1 ride Pool SWDGE (its descriptor gen overlaps the
            # head transfer); mv8 goes in quarters so it never blocks mkq.
            nc.sync.dma_start(out=head_sb[:], in_=head[:])
            nc.sync.dma_start(
                out=mkq_sb[:, : CH * P], in_=mkq[:, CH * P : 2 * CH * P]
            )
            nc.sync.dma_start(out=mv_sb[:, : 2 * CV], in_=mvp[:, :2, :])
            nc.sync.dma_start(
                out=mkq_sb[:, CH * P : 2 * CH * P], in_=mkq[:, 2 * CH * P : 3 * CH * P]
            )
            Q8 = (N8SLOT + 7) // 8  # mv8 eighth (in slots)
            vnext = 2
            for i in range(3, NCHUNK):
                k0, k1 = i * CH * P, (i + 1) * CH * P
                nc.sync.dma_start(
                    out=mkq_sb[:, k0 - CH * P : k1 - CH * P], in_=mkq[:, k0:k1]
                )
                if i == 4:
                    q1, nq1 = QBLKS[1]
                    nc.sync.dma_start(out=qkc_sb[:], in_=qkc[:, q1 : q1 + nq1])
                if i in (6, 9, 12, 15, 18, 21, 24, 27):
                    q = (6, 9, 12, 15, 18, 21, 24, 27).index(i)
                    s0 = Q8 * q
                    s1 = Q8 * (q + 1) if q < 7 else N8SLOT
                    nc.sync.dma_start(
                        out=mv8_sb[:, s0:s1, :], in_=mv8p[:, s0:s1, :]
                    )
                v1 = min(vnext + 2, NB)
                if vnext < v1:
                    nc.sync.dma_start(
                        out=mv_sb[:, vnext * CV : v1 * CV], in_=mvp[:, vnext:v1, :]
                    )
                    vnext = v1
            if vnext < NB:
                nc.sync.dma_start(
                    out=mv_sb[:, vnext * CV :], in_=mvp[:, vnext:, :]
                )

            bias_sb = cpool.tile([P, 1], f32, tag="bias", name="bias")
            nc.vector.memset(bias_sb[:], EXP_BIAS)
            ones_col_f = cpool.tile([P, 1], f32, tag="ones_col_f", name="ones_col_f")
            nc.vector.memset(ones_col_f[:], 1.0)
            ones_col = cpool.tile([P, 1], f32r, tag="ones_col", name="ones_col")
            with nc.allow_low_precision("exact 1.0 cast to f32r"):
                nc.vector.tensor_copy(ones_col[:], ones_col_f[:])
            ones_col_b = cpool.tile([P, 1], bf16, tag="ones_col_b", name="ones_col_b")
            nc.vector.memset(ones_col_b[:], 1.0)
            ones_col_h = cpool.tile([P, 1], fp16, tag="ones_col_h", name="ones_col_h")
            nc.vector.memset(ones_col_h[:], 1.0)
            ones_row_f = cpool.tile([1, P], f32, tag="ones_row_f", name="ones_row_f")
            nc.vector.memset(ones_row_f[:], 1.0)
            ones_row = cpool.tile([1, P], f32r, tag="ones_row", name="ones_row")
            with nc.allow_low_precision("exact 1.0 cast to f32r"):
                nc.vector.tensor_copy(ones_row[:], ones_row_f[:])

            # Per-block state created lazily at block entry.
            rstate = [None]  # current residual pair's w_lo buffer
            accs = [None, None]   # each: two [P, 2, 512] psum bank-pairs
            dens = [None, None]
            scores = [None] * NVT

            def vt_block(j):
                return 0 if j < NT else 1

            def emit_score(j):
                blk = vt_block(j)
                q0, nq = QBLKS[blk]
                mi = j - blk * NT
                if mi < CH:
                    lhsT = head_sb[:, NQ0 + mi * P : NQ0 + (mi + 1) * P]
                else:
                    lhsT = mkq_sb[:, (mi - CH) * P : (mi - CH + 1) * P]
                rhs = head_sb[:, :NQ0] if blk == 0 else qkc_sb[:]
                s = spspool.tile([P, nq], f32, tag="score", name="score")
                nc.tensor.matmul(s[:], lhsT=lhsT, rhs=rhs, start=True, stop=True)
                scores[j] = s

            def emit_block_entry_dens(blk):
                d = dpool.tile([P, QBLKS[blk][1]], fp16, tag="den", name="den")
                nc.vector.memset(d[:], 0.0)
                dens[blk] = d

            def emit_block_entry_accs(blk):
                accs[blk] = [
                    apspool.tile([P, 2, PSW], f32, tag=f"accp{p}", name=f"accp{p}")
                    for p in range(2)
                ]

            recips = [None, None]
            den_sums = [None, None]

            def emit_den_main(blk):
                # Partial denominator (tiles 0..NT-2) reduced on PE right
                # after the second-to-last DVE add -- off the endgame chain.
                nq = QBLKS[blk][1]
                den_sum = dpspool.tile([1, nq], f32, tag="den_sum", name="den_sum")
                nc.tensor.matmul(
                    den_sum[:],
                    lhsT=ones_col_h[:],
                    rhs=dens[blk][:],
                    start=True,
                    stop=False,
                )
                den_sums[blk] = den_sum

            def emit_den_tail_recip(blk, ex_last):
                # Last tile's exp summed straight off ACT's output on the PE,
                # skipping the last DVE accumulator add on the endgame chain.
                nq = QBLKS[blk][1]
                nc.tensor.matmul(
                    den_sums[blk][:],
                    lhsT=ones_col_b[:],
                    rhs=ex_last[:],
                    start=False,
                    stop=True,
                )
                r = vpool.tile([1, nq], f32r, tag="recip", name="recip")
                with nc.allow_low_precision("feeds f32r broadcast matmul"):
                    nc.vector.reciprocal(r[:], den_sums[blk][:])
                recips[blk] = r

            evacs = [None, None]

            def emit_evac(blk):
                # ACT evacuates the two acc bank-pairs to fp16 SBUF, freeing
                # the psum banks for the next block ~2.5us sooner than
                # normalize-in-place; normalization happens later during the
                # next block's DVE slack.
                q0, nq = QBLKS[blk]
                evacs[blk] = []
                for pair in range(2):
                    e = cpool.tile([P, 2, PSW], fp16, tag=f"evac{pair}", name=f"evac{pair}")
                    nc.scalar.copy(e[:, :, :nq], accs[blk][pair][:, :, :nq])
                    evacs[blk].append(e)

            def emit_norm_deferred(blk):
                q0, nq = QBLKS[blk]
                bcast_ps = spspool.tile([P, nq], f32, tag="score", name="bcast")
                nc.tensor.matmul(
                    bcast_ps[:],
                    lhsT=ones_row[:],
                    rhs=recips[blk][:],
                    start=True,
                    stop=True,
                )
                bcast_sb = vpool.tile([P, nq], bf16, tag="bcast_sb", name="bcast_sb")
                nc.vector.tensor_copy(bcast_sb[:], bcast_ps[:])
                rep = bcast_sb[:].unsqueeze(1).broadcast_to([P, 2, nq])
                engs = [nc.sync, nc.gpsimd]
                for pair in range(2):
                    o = opool.tile([P, 2, PSW], bf16, tag="out", name="out")
                    nc.vector.tensor_mul(
                        o[:, :, :nq], evacs[blk][pair][:, :, :nq], rep
                    )
                    dst = out[
                        pair * 2 * P : (pair + 1) * 2 * P, q0 : q0 + nq
                    ].rearrange("(two p) q -> p two q", two=2)
                    engs[pair].dma_start(out=dst, in_=o[:, :, :nq])

            def emit_bcast(blk):
                # bcast the per-query recip onto 128 partitions (PE), copy it
                # to SBUF (ACT).  Emitted BEFORE the last tile's readouts so
                # this chain overlaps them.
                q0, nq = QBLKS[blk]
                bcast_ps = spspool.tile([P, nq], f32, tag="score", name="bcast")
                nc.tensor.matmul(
                    bcast_ps[:],
                    lhsT=ones_row[:],
                    rhs=recips[blk][:],
                    start=True,
                    stop=True,
                )
                bcast_sb = vpool.tile([P, nq], bf16, tag="bcast_sb", name="bcast_sb")
                nc.scalar.copy(bcast_sb[:], bcast_ps[:])
                return bcast_sb

            def emit_muls(blk, bcast_sb):
                # Endgame: pair 1 is evacuated to fp16 by ACT in parallel
                # with pair 0's direct psum mul on DVE; pair 1's mul then
                # runs in DVE 2x mode (all-2-byte), shortening the serial
                # mul chain by ~375ns.
                q0, nq = QBLKS[blk]
                rep = bcast_sb[:].unsqueeze(1).broadcast_to([P, 2, nq])
                e = cpool.tile([P, 2, PSW], fp16, tag="evac1", name="evac1f")
                nc.scalar.copy(e[:, :, :nq], accs[blk][1][:, :, :nq])
                o0 = opool.tile([P, 2, PSW], bf16, tag="out", name="out")
                nc.vector.tensor_mul(o0[:, :, :nq], accs[blk][0][:, :, :nq], rep)
                o1 = opool.tile([P, 2, PSW], bf16, tag="out", name="out")
                nc.vector.tensor_mul(o1[:, :, :nq], e[:, :, :nq], rep)
                for pair, o in ((0, o0), (1, o1)):
                    dst = out[
                        pair * 2 * P : (pair + 1) * 2 * P, q0 : q0 + nq
                    ].rearrange("(two p) q -> p two q", two=2)
                    nc.sync.dma_start(out=dst, in_=o[:, :, :nq])

            def _dr4(blk, nq, base, rhs, start, stop):
                for c in range(4):
                    nc.tensor.matmul(
                        accs[blk][c // 2][:, c % 2, :nq],
                        lhsT=mv8_sb[:, base : base + 2, c * P : (c + 1) * P],
                        rhs=rhs,
                        start=start,
                        stop=stop,
                        perf_mode=DR,
                    )

            def emit_readout(j, ex, whi=None, wlo=None):
                blk = vt_block(j)
                mi = j - blk * NT
                nq = QBLKS[blk][1]
                kind = KIND_BY_TILE[mi]
                sl = _SLOT[mi]
                if kind == "B":
                    def _ro(blk=blk, nq=nq, sl=sl, mi=mi, ex=ex):
                        for c in range(4):
                            nc.tensor.matmul(
                                accs[blk][c // 2][:, c % 2, :nq],
                                lhsT=mv_sb[
                                    :, sl * CV + c * P : sl * CV + (c + 1) * P
                                ],
                                rhs=ex[:],
                                start=(mi == 0),
                                stop=(mi == NT - 1),
                            )
                elif kind == "N":
                    def _ro(blk=blk, nq=nq, sl=sl, mi=mi, ex=ex):
                        rhs = ex[:].unsqueeze(1).broadcast_to([P, 2, nq])
                        _dr4(blk, nq, sl, rhs, mi == 0, mi == NT - 1)
                else:
                    def _ro(blk=blk, nq=nq, sl=sl, whi=whi):
                        rhs = whi[:].unsqueeze(1).broadcast_to([P, 2, nq])
                        _dr4(blk, nq, sl, rhs, False, False)
                    if not _RFIRST[mi]:
                        # second of pair: the cross instr (hiA, hiB) x
                        # (w_lo_a, w_lo_b) waits on the DVE lo-split; give it
                        # one more iteration of slack.
                        def _rx(blk=blk, nq=nq, sl=sl, wlo=wlo):
                            _dr4(blk, nq, sl - 1, wlo[:, :, :nq], False, False)
                        pending.append((j + 3, _rx))
                pending.append((j + 2, _ro))

            emit_block_entry_dens(0)
            emit_block_entry_accs(0)
            emit_score(0)
            emit_score(1)
            pending = []
            for j in range(NVT):
                if j == NT:
                    for jj in range(j + 2, j + 6):
                        emit_score(jj)
                elif j + 2 < NVT and not (NT < j < NT + 4):
                    emit_score(j + 2)
                if j == NT + 1:
                    # block 1's acc psum rotates in only now: every access to
                    # block 0's acc tiles has been emitted.
                    emit_block_entry_accs(1)
                for item in pending[:]:
                    if item[0] <= j:
                        item[1]()
                        pending.remove(item)
                blk = vt_block(j)
                if j == NT:
                    emit_block_entry_dens(1)
                q0, nq = QBLKS[blk]
                mi = j - blk * NT
                if mi == NT - 1 and j == NVT - 1:
                    # exp/den-tail/recip were hoisted into the previous
                    # iteration: flush r/o[NVT-2], then bcast, then the last
                    # readouts inline so the bcast chain overlaps them.
                    for item in pending:
                        item[1]()
                    pending.clear()
                    final_bcast = emit_bcast(1)
                    for c in range(4):
                        nc.tensor.matmul(
                            accs[blk][c // 2][:, c % 2, :nq],
                            lhsT=mv_sb[
                                :,
                                _SLOT[mi] * CV + c * P : _SLOT[mi] * CV
                                + (c + 1) * P,
                            ],
                            rhs=hoisted_ex[:],
                            start=False,
                            stop=True,
                        )
                    continue
                kind = KIND_BY_TILE[mi]
                ex = (
                    ex8pool.tile([P, nq], fp8, tag="exp8", name="exp8")
                    if kind == "N"
                    else expool.tile([P, nq], bf16, tag="exp", name="exp")
                )
                nc.scalar.activation(
                    ex[:], scores[j][:], Exp, bias=bias_sb[:], scale=0.25
                )
                scores[j] = None
                whi = wlo = None
                if kind == "R":
                    whi = ex8pool.tile([P, nq], fp8, tag="exp8", name="whi")
                    if _RFIRST[mi]:
                        rstate[0] = wlopool.tile(
                            [P, 2, PSW], fp8, tag="wlo", name="wlo"
                        )
                    wlo = rstate[0]
                    plane = 0 if _RFIRST[mi] else 1
                    with nc.allow_low_precision("fp8 exp hi/lo split"):
                        nc.vector.tensor_copy(whi[:], ex[:])
                        nc.vector.scalar_tensor_tensor(
                            wlo[:, plane, :nq],
                            ex[:],
                            1.0,
                            whi[:],
                            mybir.AluOpType.mult,
                            mybir.AluOpType.subtract,
                        )
                if mi < NT - 1:
                    with nc.allow_low_precision("den feeds f32r den_sum matmul"):
                        nc.vector.tensor_add(dens[blk][:], dens[blk][:], ex[:])
                    if mi == NT - 2:
                        emit_den_main(blk)
                        if j == NVT - 2:
                            # Hoist the final tile's exp + den tail + recip so
                            # the bcast chain overlaps the last 8 readouts.
                            hoisted_ex = expool.tile(
                                [P, nq], bf16, tag="exp", name="exp"
                            )
                            nc.scalar.activation(
                                hoisted_ex[:],
                                scores[NVT - 1][:],
                                Exp,
                                bias=bias_sb[:],
                                scale=0.25,
                            )
                            scores[NVT - 1] = None
                            emit_den_tail_recip(blk, hoisted_ex)
                else:
                    emit_den_tail_recip(blk, ex)
                if j == NT:
                    # Block 1's outputs: flush block 0's remaining readouts,
                    # evacuate (fast bank free), normalize later during block
                    # 1's slack.
                    for item in pending:
                        item[1]()
                    pending.clear()
                    emit_evac(0)
                if j == NT + 8:
                    emit_norm_deferred(0)
                emit_readout(j, ex, whi, wlo)
            emit_muls(1, final_bcast)

    nc.compile()
    return nc


def _get_program():
    global _PROGRAM
    if _PROGRAM is None:
        _PROGRAM = _build_program()
    return _PROGRAM


def _make_in_maps(mk, qk, mv):
    mkf = np.asarray(mk, dtype=np.float32).reshape(B, CK, THW)
    qkf = np.asarray(qk, dtype=np.float32).reshape(B, CK, NQ)
    mvf = np.asarray(mv, dtype=np.float32).reshape(B, CV, THW)

    bf_idx = [t for t in range(NT) if KIND_BY_TILE[t] == "B"]

    in_maps = []
    for b in range(B):
        mkq_b = np.zeros((P, THWP), dtype=_BF16)
        mkq_b[:CK, :THW] = mkf[b]
        mkq_b[CK:, :THW] = mkf[b] * mkf[b]
        mkq_b[CK, THW:] = PAD_POISON  # pad tokens -> logit -80 -> exp ~ 0

        mvt = np.zeros((THWP, CV), dtype=np.float32)
        mvt[:THW] = mvf[b].T
        mvp_full = np.ascontiguousarray(mvt.reshape(NT, P, CV).transpose(1, 0, 2))
        mvp_b = mvp_full[:, bf_idx, :].astype(_BF16)
        hi_all = mvp_full.astype(_FP8)
        lo_all = (mvp_full - hi_all.astype(np.float32)).astype(_FP8)
        mv8_b = np.zeros((P, N8SLOT, CV), dtype=_FP8)
        for t in range(NT):
            k = KIND_BY_TILE[t]
            s = _SLOT[t]
            if k == "N":
                mv8_b[:, s] = hi_all[:, t]
                mv8_b[:, s + 1] = lo_all[:, t]
            elif k == "R" and _RFIRST[t]:
                mv8_b[:, s] = hi_all[:, t]      # hiA
                mv8_b[:, s + 1] = lo_all[:, t]  # loA
                mv8_b[:, s + 2] = hi_all[:, t]  # hiA dup (cross instr)
            elif k == "R":
                mv8_b[:, s] = hi_all[:, t]      # hiB (slot base+3)
                mv8_b[:, s + 1] = lo_all[:, t]  # loB

        for h in range(2):
            qkc_b = np.empty((P, QH), dtype=_BF16)
            qkc_b[:CK] = qkf[b][:, h * QH : (h + 1) * QH]
            qkc_b[CK:] = -0.5
            nq0 = QBLKS[0][1]
            head_b = np.concatenate([qkc_b[:, :nq0], mkq_b[:, : CH * P]], axis=1)
            in_maps.append(
                {
                    "mkq": mkq_b,
                    "qkc": qkc_b,
                    "mvp": mvp_b,
                    "mv8p": mv8_b,
                    "head": head_b,
                }
            )
    return in_maps


def kernel(mk, qk, mv, _trace=False, _results_out=None):
    from concourse import bass_utils

    nc = _get_program()
    in_maps = _make_in_maps(mk, qk, mv)
    res = bass_utils.run_bass_kernel_spmd(
        nc, in_maps, core_ids=list(range(8)), trace=_trace
    )
    if _results_out is not None:
        _results_out.append(res)

    full = np.empty((B, CV, NQ), dtype=np.float32)
    for b in range(B):
        for h in range(2):
            full[b][:, h * QH : (h + 1) * QH] = np.asarray(
                res.results[2 * b + h]["out"], dtype=np.float32
            )
    return full.reshape(B, CV, H, W)
